# revision 4
# baseline (speedup 1.0000x reference)
"""Trainium2 kernel for nn_PointerSelectorV4_7 (topk_masking).

Sharding: pure data parallel over batch B across 8 NeuronCores.

Device computes the heavy candidate projections (83% of FLOPs):
    c   = cand_emb @ W_in.T        [B,K,768]@[768,256]
    cc  = c @ Wc.T                 [B,K,256]@[256,256]
    cdc = c @ (W_dpp @ Wc).T       [B,K,256]@[256,32]
in feature-major layout via 3-pass bf16 hi/lo split matmuls (near-fp32
accuracy: hi@Whi + hi@Wlo + lo@Whi, fp32 PSUM accumulation).

Host pre-transposes cand_emb (so the contraction dim lands on SBUF
partitions with contiguous DMA) and finishes the cheap sequential shot
loop / attention / loss in numpy.
"""
import numpy as np
import ml_dtypes

B, K, D, H, R, S = 8192, 32, 768, 256, 32, 4
NCORES = 8
BC = B // NCORES            # batch rows per core
NC_ROWS = BC * K            # candidate rows per core (32768)
MAX_SHOT = 8
TEMP = 0.1
LS = 0.1
CHUNK = 512                 # candidate columns per matmul chunk
NCHUNK = NC_ROWS // CHUNK   # 64

bf16 = ml_dtypes.bfloat16

_compiled = None
LAST_EXEC_NS = None


def _split(x):
    hi = np.asarray(x, np.float32).astype(bf16)
    lo = (np.asarray(x, np.float32) - hi.astype(np.float32)).astype(bf16)
    return hi, lo


def _build():
    import concourse.bacc as bacc
    import concourse.mybir as mybir
    from concourse import tile

    nc = bacc.Bacc("TRN2", target_bir_lowering=False, debug=False)
    dt = mybir.dt

    # DRAM I/O (per-core shard)
    x_hi_d = nc.dram_tensor("x_hi", [D, NC_ROWS], dt.bfloat16, kind="ExternalInput").ap()
    x_lo_d = nc.dram_tensor("x_lo", [D, NC_ROWS], dt.bfloat16, kind="ExternalInput").ap()
    win_hi_d = nc.dram_tensor("win_hi", [D, H], dt.bfloat16, kind="ExternalInput").ap()
    win_lo_d = nc.dram_tensor("win_lo", [D, H], dt.bfloat16, kind="ExternalInput").ap()
    wc_hi_d = nc.dram_tensor("wc_hi", [H, H], dt.bfloat16, kind="ExternalInput").ap()
    wc_lo_d = nc.dram_tensor("wc_lo", [H, H], dt.bfloat16, kind="ExternalInput").ap()
    wdc_hi_d = nc.dram_tensor("wdc_hi", [H, R], dt.bfloat16, kind="ExternalInput").ap()
    wdc_lo_d = nc.dram_tensor("wdc_lo", [H, R], dt.bfloat16, kind="ExternalInput").ap()
    ct_hi_d = nc.dram_tensor("ct_hi", [H, NC_ROWS], dt.bfloat16, kind="ExternalOutput").ap()
    ct_lo_d = nc.dram_tensor("ct_lo", [H, NC_ROWS], dt.bfloat16, kind="ExternalOutput").ap()
    cct_d = nc.dram_tensor("cct", [H, NC_ROWS], dt.float32, kind="ExternalOutput").ap()
    cdct_d = nc.dram_tensor("cdct", [R, NC_ROWS], dt.float32, kind="ExternalOutput").ap()

    KT = D // 128  # 6

    with tile.TileContext(nc) as tc:
        with (
            tc.tile_pool(name="w", bufs=1) as wp,
            tc.tile_pool(name="xs", bufs=3) as xp,
            tc.tile_pool(name="cs", bufs=3) as cp,
            tc.tile_pool(name="os", bufs=3) as op,
            tc.tile_pool(name="ps", bufs=2, space="PSUM") as pp,
        ):
            # --- weights resident ---
            win_hi = wp.tile([128, KT, H], dt.bfloat16, tag="win_hi")
            win_lo = wp.tile([128, KT, H], dt.bfloat16, tag="win_lo")
            wc_hi = wp.tile([128, 2, H], dt.bfloat16, tag="wc_hi")
            wc_lo = wp.tile([128, 2, H], dt.bfloat16, tag="wc_lo")
            wdc_hi = wp.tile([128, 2, R], dt.bfloat16, tag="wdc_hi")
            wdc_lo = wp.tile([128, 2, R], dt.bfloat16, tag="wdc_lo")
            nc.gpsimd.dma_start(win_hi[:], win_hi_d.rearrange("(kt p) h -> p kt h", p=128))
            nc.gpsimd.dma_start(win_lo[:], win_lo_d.rearrange("(kt p) h -> p kt h", p=128))
            nc.gpsimd.dma_start(wc_hi[:], wc_hi_d.rearrange("(kt p) h -> p kt h", p=128))
            nc.gpsimd.dma_start(wc_lo[:], wc_lo_d.rearrange("(kt p) h -> p kt h", p=128))
            nc.gpsimd.dma_start(wdc_hi[:], wdc_hi_d.rearrange("(kt p) r -> p kt r", p=128))
            nc.gpsimd.dma_start(wdc_lo[:], wdc_lo_d.rearrange("(kt p) r -> p kt r", p=128))

            for ch in range(NCHUNK):
                c0 = ch * CHUNK
                # --- load x chunk (transposed cand_emb, hi/lo) ---
                xh = xp.tile([128, KT, CHUNK], dt.bfloat16, tag="xh")
                xl = xp.tile([128, KT, CHUNK], dt.bfloat16, tag="xl")
                nc.sync.dma_start(
                    xh[:], x_hi_d.rearrange("(kt p) n -> p kt n", p=128)[:, :, c0:c0 + CHUNK])
                nc.sync.dma_start(
                    xl[:], x_lo_d.rearrange("(kt p) n -> p kt n", p=128)[:, :, c0:c0 + CHUNK])

                # --- c = W_in.T.T @ x : 3-pass bf16, out [256, CHUNK] ---
                ch_t = cp.tile([128, 2, CHUNK], dt.bfloat16, tag="c_hi")
                cl_t = cp.tile([128, 2, CHUNK], dt.bfloat16, tag="c_lo")
                for mt in range(2):
                    ps = pp.tile([128, CHUNK], dt.float32, tag="c_ps")
                    first = True
                    for wl, xx in ((win_hi, xh), (win_lo, xh), (win_hi, xl)):
                        for kt in range(KT):
                            nc.tensor.matmul(
                                ps[:], wl[:, kt, bass_ts(mt, 128)], xx[:, kt, :],
                                start=first, stop=(wl is win_hi and xx is xl and kt == KT - 1))
                            first = False
                    # split copy-out: hi (ACT) then lo = psum - hi (DVE)
                    nc.scalar.copy(ch_t[:, mt, :], ps[:])
                    nc.vector.tensor_tensor(
                        cl_t[:, mt, :], ps[:], ch_t[:, mt, :], mybir.AluOpType.subtract)
                nc.sync.dma_start(ct_hi_d[0:128, c0:c0 + CHUNK], ch_t[:, 0, :])
                nc.sync.dma_start(ct_hi_d[128:256, c0:c0 + CHUNK], ch_t[:, 1, :])
                nc.sync.dma_start(ct_lo_d[0:128, c0:c0 + CHUNK], cl_t[:, 0, :])
                nc.sync.dma_start(ct_lo_d[128:256, c0:c0 + CHUNK], cl_t[:, 1, :])

                # --- cc = Wc.T.T @ c : 3-pass from on-chip c hi/lo ---
                cc_sb = op.tile([128, 2, CHUNK], dt.float32, tag="cc_sb")
                for mt in range(2):
                    ps2 = pp.tile([128, CHUNK], dt.float32, tag="cc_ps")
                    first = True
                    for wl, xx in ((wc_hi, ch_t), (wc_lo, ch_t), (wc_hi, cl_t)):
                        for kt in range(2):
                            nc.tensor.matmul(
                                ps2[:], wl[:, kt, bass_ts(mt, 128)], xx[:, kt, :],
                                start=first, stop=(wl is wc_hi and xx is cl_t and kt == 1))
                            first = False
                    nc.scalar.copy(cc_sb[:, mt, :], ps2[:])
                nc.sync.dma_start(cct_d[0:128, c0:c0 + CHUNK], cc_sb[:, 0, :])
                nc.sync.dma_start(cct_d[128:256, c0:c0 + CHUNK], cc_sb[:, 1, :])

                # --- cdc = (W_dpp@Wc).T.T @ c : 3-pass, out [32, CHUNK] ---
                cdc_sb = op.tile([128, CHUNK], dt.float32, tag="cdc_sb")
                ps3 = pp.tile([32, CHUNK], dt.float32, tag="cdc_ps")
                first = True
                for wl, xx in ((wdc_hi, ch_t), (wdc_lo, ch_t), (wdc_hi, cl_t)):
                    for kt in range(2):
                        nc.tensor.matmul(
                            ps3[:], wl[:, kt, :], xx[:, kt, :],
                            start=first, stop=(wl is wdc_hi and xx is cl_t and kt == 1))
                        first = False
                nc.scalar.copy(cdc_sb[0:32, :], ps3[:])
                nc.sync.dma_start(cdct_d[:, c0:c0 + CHUNK], cdc_sb[0:32, :])

    nc.compile()
    return nc


def bass_ts(i, n):
    import concourse.bass as bass
    return bass.ts(i, n)


def _run_device(cand_emb, W_in, Wc, Wdc):
    """Returns full-batch c, cc, cdc as float32 [B,K,*]."""
    import os
    os.environ["BASS_NEVER_TRACE"] = "1"  # axon NTFF hook unavailable here
    from concourse.bass_utils import run_bass_kernel_spmd
    global _compiled
    if _compiled is None:
        _compiled = _build()
    nc = _compiled

    xT = np.ascontiguousarray(cand_emb.reshape(B * K, D).T)      # [768, B*K]
    xT_hi, xT_lo = _split(xT)
    win_hi, win_lo = _split(W_in.T)                               # lhsT [768,256]
    wc_hi, wc_lo = _split(Wc.T)                                   # lhsT [256,256]
    wdc_hi, wdc_lo = _split(Wdc.T)                                # lhsT [256,32]

    in_maps = []
    for core in range(NCORES):
        s = slice(core * NC_ROWS, (core + 1) * NC_ROWS)
        in_maps.append({
            "x_hi": np.ascontiguousarray(xT_hi[:, s]),
            "x_lo": np.ascontiguousarray(xT_lo[:, s]),
            "win_hi": win_hi, "win_lo": win_lo,
            "wc_hi": wc_hi, "wc_lo": wc_lo,
            "wdc_hi": wdc_hi, "wdc_lo": wdc_lo,
        })
    res = run_bass_kernel_spmd(nc, in_maps, core_ids=list(range(NCORES)))
    global LAST_EXEC_NS
    LAST_EXEC_NS = res.exec_time_ns
    c = np.empty((B * K, H), np.float32)
    cc = np.empty((B * K, H), np.float32)
    cdc = np.empty((B * K, R), np.float32)
    for core, out in enumerate(res.results):
        s = slice(core * NC_ROWS, (core + 1) * NC_ROWS)
        c[s] = (out["ct_hi"].astype(np.float32) + out["ct_lo"].astype(np.float32)).T
        cc[s] = out["cct"].T
        cdc[s] = out["cdct"].T
    return c.reshape(B, K, H), cc.reshape(B, K, H), cdc.reshape(B, K, R)


def _l2n(x):
    n = np.linalg.norm(x, axis=-1, keepdims=True)
    return x / np.maximum(n, 1e-12)


def kernel(query_emb, cand_emb, labels, W_in, attn_in_w, attn_in_b,
           attn_out_w, attn_out_b, ln_g, ln_b, Wq, Wc,
           gru_wih, gru_whh, gru_bih, gru_bhh, step_emb, W_dpp, dpp_lambda):
    query_emb = np.asarray(query_emb, np.float32)
    cand_emb = np.asarray(cand_emb, np.float32)
    labels_in = np.asarray(labels)
    W_in = np.asarray(W_in, np.float32)
    Wc = np.asarray(Wc, np.float32)
    Wdc = (np.asarray(W_dpp, np.float32) @ Wc).astype(np.float32)

    # ---- device: heavy candidate projections (data parallel over B) ----
    c, cc, cdc = _run_device(cand_emb, W_in, Wc, Wdc)

    # ---- host: cheap sequential remainder (numpy, fp32) ----
    q = query_emb @ W_in.T                                        # [B,H]
    Wqp, Wkp, Wvp = np.split(np.asarray(attn_in_w, np.float32), 3, axis=0)
    bq, bk, bv = np.split(np.asarray(attn_in_b, np.float32), 3)
    qh = q @ Wqp.T + bq
    qk = qh @ Wkp                                                  # pulled-through keys
    scores = ((c @ qk[..., None])[..., 0] + (qh @ bk)[:, None]) / np.sqrt(np.float32(H))
    scores -= scores.max(-1, keepdims=True)
    e = np.exp(scores)
    att = e / e.sum(-1, keepdims=True)
    cbar = (att[:, None, :] @ c)[:, 0]
    ao = (cbar @ Wvp.T + bv) @ np.asarray(attn_out_w, np.float32).T + np.asarray(attn_out_b, np.float32)
    x = ao + q
    mu = x.mean(-1, keepdims=True)
    var = x.var(-1, keepdims=True)
    enh = (x - mu) / np.sqrt(var + 1e-5) * np.asarray(ln_g, np.float32) + np.asarray(ln_b, np.float32)
    h = _l2n(enh @ np.asarray(Wq, np.float32).T)

    nc_ = np.linalg.norm(cc, axis=-1)                              # [B,K]
    nc_s = np.maximum(nc_, 1e-12)
    cand_n = cc / nc_s[..., None]
    dpp = _l2n(cdc)
    lam = np.log1p(np.exp(np.float32(dpp_lambda))).astype(np.float32)
    se = np.asarray(step_emb, np.float32)
    wih = np.asarray(gru_wih, np.float32)
    whh = np.asarray(gru_whh, np.float32)
    bih = np.asarray(gru_bih, np.float32)
    bhh = np.asarray(gru_bhh, np.float32)
    lab = labels_in.astype(np.int64)

    mask = np.zeros((B, K), bool)
    max_sim = np.full((B, K), -np.inf, np.float32)
    logits_list = []
    for s in range(S):
        he = _l2n(h + se[min(s, MAX_SHOT - 1)])
        base = (c @ (he @ Wc)[..., None])[..., 0] / nc_s / np.float32(TEMP)
        if s > 0:
            gain = np.log(1e-6 + 1.0 - np.minimum(max_sim ** 2, 0.999))
            sc = base + lam * gain
        else:
            sc = base
        sc = np.where(mask, np.float32(-100.0), sc).astype(np.float32)
        logits_list.append(sc)
        idx = lab[:, s]
        mask = mask | (np.arange(K)[None, :] == idx[:, None])
        chosen = np.take_along_axis(cand_n, idx[:, None, None], axis=1)[:, 0]
        chosen_dpp = np.take_along_axis(dpp, idx[:, None, None], axis=1)[:, 0]
        max_sim = np.maximum(max_sim, (dpp @ chosen_dpp[..., None])[..., 0])
        gi = chosen @ wih.T + bih
        gh = h @ whh.T + bhh
        ir, iz, inn = np.split(gi, 3, -1)
        hr, hz, hn = np.split(gh, 3, -1)
        r = 1.0 / (1.0 + np.exp(-(ir + hr)))
        z = 1.0 / (1.0 + np.exp(-(iz + hz)))
        n = np.tanh(inn + r * hn)
        h = _l2n((1.0 - z) * n + z * h)

    logits = np.stack(logits_list, axis=1).astype(np.float32)      # [B,S,K]
    predictions = np.argmax(logits, axis=-1).astype(np.int32)      # [B,S]
    lg = np.maximum(logits.reshape(-1, K), np.float32(-100.0))
    m = lg.max(-1, keepdims=True)
    logp = lg - m - np.log(np.exp(lg - m).sum(-1, keepdims=True))
    nll = -np.take_along_axis(logp, lab.reshape(-1)[:, None], axis=1)[:, 0]
    smooth = -logp.mean(-1)
    loss = np.float32(((1.0 - LS) * nll + LS * smooth).mean())
    return logits, predictions, loss
